# revision 1
# baseline (speedup 1.0000x reference)
"""Multi-head causal attention on 8 Trainium2 NeuronCores.

Sharding: tensor-parallel over heads (2 heads/core) for QKV projection and
attention; All-to-All converts to token-sharding (512 tokens/core) for the
output projection, so each core writes a disjoint output slice and the host
gather is pure concatenation.

Layout strategy (per core):
  - qkvT = Wqkv_shard^T @ x^T computed directly in transposed [feature, token]
    layout (x^T prepared on host) so the attention contractions need no
    on-chip transposes of Q/K.
  - scores^T[k, q] = K_tile^T.T @ Q^T with the two heads row-tiled on the PE
    (d=64 contraction each, partitions 0-63 / 64-127, concurrent).
  - softmax: exp on ScalarE straight out of PSUM with the 1/sqrt(D) scale
    folded into the activation's free affine; no max-subtraction (scores are
    O(6) here so exp is safe in fp32/bf16); the denominator comes for free as
    a ones-column appended to V in the AV matmul; causal masking is a
    multiplicative 0/1 bf16 mask on the diagonal tiles only.
  - AV: av^T[d, q] accumulated over k-tiles in PSUM; lhsT = [V_k | 1] needs V
    in natural [token, d] layout, produced by xbar DMA-transpose through a
    DRAM bounce.
  - normalize: 1/denom broadcast across partitions via a DRAM-replicate DMA,
    single fused DVE multiply writes the bf16 A2A payload.
"""

import numpy as np
import ml_dtypes

import concourse.bass as bass
import concourse.mybir as mybir
import concourse.tile as tile
from concourse.bass_utils import run_bass_kernel_spmd
from concourse.masks import make_identity
from concourse.vector_clock import ScopedClock

F32 = mybir.dt.float32
BF16 = mybir.dt.bfloat16
AF = mybir.ActivationFunctionType


def _install_cache_nonce_hook():
    """The libneuronxla NEFF cache hashes the HLO but the BIR rides in
    backend_config (excluded from the hash), so edited kernels with the same
    I/O signature can silently hit a stale cached NEFF. Inject a hash of the
    BIR into mhlo.frontend_attributes — which IS part of the model hash —
    the same way bass2jax ships the DVE tables."""
    import hashlib
    import concourse.bass2jax as bass2jax
    from jax.interpreters import mlir

    if getattr(bass2jax, "_ant_cache_nonce_hooked", False):
        return
    bass2jax._ant_cache_nonce_hooked = True
    orig = bass2jax._accumulate_module_dve_attrs

    def patched(ctx, nc):
        orig(ctx, nc)
        op = ctx.module_context.module.operation
        cur = (
            op.attributes["mhlo.frontend_attributes"]
            if "mhlo.frontend_attributes" in op.attributes
            else None
        )
        existing = (
            {a.name: mlir.ir.StringAttr(a.attr).value for a in cur}
            if cur is not None
            else {}
        )
        existing["ant.cache_nonce"] = hashlib.sha256(
            nc.to_json_bytes()
        ).hexdigest()
        op.attributes["mhlo.frontend_attributes"] = mlir.ir.DictAttr.get(
            {k: mlir.ir.StringAttr.get(v) for k, v in existing.items()}
        )

    bass2jax._accumulate_module_dve_attrs = patched


_install_cache_nonce_hook()


def _install_ldw_opt_hook():
    """bass_utils hardcodes --enable-ldw-opt=false; with it, walrus emits a
    serialized LDWEIGHTS before every MATMUL (~40% PE overhead here). Flip
    the flag on the walrus command line."""
    import concourse.bass_utils as bu

    if getattr(bu, "_ant_ldw_opt_hooked", False):
        return
    bu._ant_ldw_opt_hooked = True
    orig = bu.run_command

    def patched(argv, **kwargs):
        argv = [
            "--enable-ldw-opt=true" if a == "--enable-ldw-opt=false" else a
            for a in argv
        ]
        return orig(argv, **kwargs)

    bu.run_command = patched


# _install_ldw_opt_hook()  # breaks walrus visitInstLdweights codegen for f32 weights

B, S, DM = 2, 2048, 1024
H, D = 16, 64
NCORES = 8
HP = H // NCORES          # heads per core
T = B * S                 # 4096 tokens
TCHUNK = T // NCORES      # 512 tokens per a2a chunk
NCH = T // 512            # 8 token chunks of 512
KT_PER_S = S // 128       # 16 k-tiles per sequence
QT_PER_S = S // 512       # 4 q-tiles per sequence
SCALE = 1.0 / np.sqrt(D)


MAX_WAITS = 1  # walrus in this container rejects >1 sem-wait per instruction


def _split_waits(nc, limit=MAX_WAITS):
    """Post-pass: move excess sem-waits onto preceding same-engine nops.

    Engines dispatch in program order and a sem-wait stalls the engine's NX
    before anything later is enqueued, so carrying the waits on nops placed
    immediately before the instruction is semantically identical.
    """
    n_id = 0
    for bb in nc.main_func.blocks:
        new = []
        for inst in bb.instructions:
            si = getattr(inst, "sync_info", None)
            if si is not None and len(si.on_wait) > limit:
                waits = list(si.on_wait)
                for i in range(0, len(waits) - limit, limit):
                    nop = mybir.InstNoOp(
                        name=f"wsplit-{n_id}", ins=[], outs=[], engine=inst.engine
                    )
                    n_id += 1
                    nop.sync_info = mybir.SyncInfo(
                        on_wait=waits[i : i + limit], on_update=[]
                    )
                    new.append(nop)
                kept = waits[len(waits) - limit :]
                inst.sync_info = mybir.SyncInfo(
                    on_wait=kept, on_update=list(si.on_update)
                )
            new.append(inst)
        bb.instructions = new


class _TileCtx(tile.TileContext):
    """Work around a walrus codegen limit: the stock tail drain carries one
    sem-wait per (engine, DMA-lane), but this compiler build rejects >1-2
    waits on a Drain ("Too many sync wait commands"). Put each wait on its
    own SP nop between the drain and the final barrier instead."""

    def _drain_and_barrier(self, tick_clock, wait_clock):
        nc = self.nc
        drain_inst = nc.sync.drain()
        wait_clock.add_sem_waits(
            drain_inst.ins, ScopedClock({None: tick_clock.global_clock})
        )
        si = drain_inst.ins.sync_info
        if si is not None and len(si.on_wait) > 1:
            waits = list(si.on_wait)
            drain_inst.ins.sync_info = mybir.SyncInfo(
                on_wait=[waits[0]], on_update=list(si.on_update)
            )
            for w in waits[1:]:
                nop = nc.sync.nop(nofuse=True, hint="tail_drain_wait_split")
                nop.ins.sync_info = mybir.SyncInfo(on_wait=[w], on_update=[])

        nc.all_engine_barrier()
        assert self.sems is not None
        popped = nc._tile_sem_poison_stack.pop()
        assert popped is self._sem_poison
        nc.clear_and_free_semaphores(list(self.sems.allocated().values()))
        nc.all_engine_barrier()


def _nkt(qt, mode):
    """Number of k-tiles attended by q-tile qt (within one sequence)."""
    return 4 * (qt + 1) if mode == "causal" else KT_PER_S


def build(mode, n_mask_tiles, skip_phase3=False):
    """Build the SPMD Bass program. mode: 'causal' | 'full' | 'general'."""
    nc = bass.Bass()
    F32R = mybir.dt.float32r

    xT = nc.dram_tensor("xT", [DM, T], F32R, kind="ExternalInput")
    wq = nc.dram_tensor("wq", [DM, 128], F32R, kind="ExternalInput")
    wk = nc.dram_tensor("wk", [DM, 128], F32R, kind="ExternalInput")
    wv = nc.dram_tensor("wv", [DM, 128], F32R, kind="ExternalInput")
    wo = nc.dram_tensor("wo", [DM, DM], F32R, kind="ExternalInput")
    if n_mask_tiles:
        mt = nc.dram_tensor(
            "mt", [n_mask_tiles, 128, 512], BF16, kind="ExternalInput"
        )
    out = nc.dram_tensor("out", [TCHUNK, DM], F32, kind="ExternalOutput")

    with _TileCtx(nc) as tc:
        with (
            tc.tile_pool(name="const", bufs=1) as const,
            tc.tile_pool(name="xin", bufs=2) as xin,
            tc.tile_pool(name="stage", bufs=3) as stage,
            tc.tile_pool(name="pp", bufs=6) as pp,
            tc.tile_pool(name="misc", bufs=4) as misc,
            tc.tile_pool(name="ps512", bufs=4, space="PSUM") as ps512,
            tc.tile_pool(name="ps1024", bufs=2, space="PSUM") as ps1024,
            tc.tile_pool(name="dram", bufs=1, space="DRAM") as dram,
        ):
            # ---- resident SBUF tensors (all loads via HWDGE, no casts) ----
            wq_sb = const.tile([128, 8, 128], F32R)
            wk_sb = const.tile([128, 8, 128], F32R)
            wv_sb = const.tile([128, 8, 128], F32R)
            nc.sync.dma_start(wq_sb[:], wq.rearrange("(o p) e -> p o e", p=128))
            nc.sync.dma_start(wk_sb[:], wk.rearrange("(o p) e -> p o e", p=128))
            nc.sync.dma_start(wv_sb[:], wv.rearrange("(o p) e -> p o e", p=128))

            if n_mask_tiles:
                mt_sb = const.tile([128, n_mask_tiles, 512], BF16)
                nc.sync.dma_start(mt_sb[:], mt.rearrange("m p q -> p m q"))

            qT_sb = const.tile([128, NCH, 512], BF16)
            kT_sb = const.tile([128, NCH, 512], BF16)
            # V in [token, feature] layout, per k-tile, per head:
            # [p=token%128, ktile, head, 80] where cols 0:64 = v, col 64 = 1.0
            # (80 keeps each (ktile, head) block 32B-aligned for the xbar)
            v_sb = const.tile([128, T // 128, HP, 80], BF16)
            nc.vector.memset(v_sb[:, :, :, 64:65], 1.0)
            ident = const.tile([128, 128], F32)
            make_identity(nc, ident[:])
            attn_all = [None, None]

            a2a_in = [
                dram.tile([NCORES, 128, 256], F32R, name=f"a2a_in{b}")
                for b in range(B)
            ]
            a2a_out = [
                dram.tile([NCORES, 128, 256], F32R, name=f"a2a_out{b}")
                for b in range(B)
            ]
            r_dram = dram.tile([NCH, HP, 1, 512], F32)

            def qkv_chunk(c):
                xt = xin.tile([128, 8, 512], F32R, tag="xt", name=f"xt{c}")
                nc.sync.dma_start(
                    xt[:],
                    xT[:, 512 * c : 512 * (c + 1)].rearrange(
                        "(o p) s -> p o s", p=128
                    ),
                )
                for name, w_sb, dst in (
                    ("q", wq_sb, qT_sb),
                    ("k", wk_sb, kT_sb),
                    ("v", wv_sb, None),
                ):
                    psum = ps512.tile(
                        [128, 512], F32, tag="ps512", name=f"ps_{name}{c}"
                    )
                    for kt in range(8):
                        nc.tensor.matmul(
                            psum[:],
                            w_sb[:, kt, :],
                            xt[:, kt, :],
                            start=(kt == 0),
                            stop=(kt == 7),
                        )
                    if dst is not None:
                        nc.vector.tensor_copy(dst[:, c, :], psum[:])
                    else:
                        # PE-based transpose of V into [token, feature]
                        # layout (the xbar DMA-transpose instruction's writes
                        # are not dependency-tracked by this Tile build).
                        vstg = stage.tile([128, 512], F32, tag="vstg")
                        nc.vector.tensor_copy(vstg[:], psum[:])
                        ps_t = ps512.tile(
                            [128, 512], F32, tag="ps512", name=f"ps_t{c}"
                        )
                        for sub in range(4):
                            nc.tensor.transpose(
                                ps_t[:, 128 * sub : 128 * (sub + 1)],
                                vstg[:, 128 * sub : 128 * (sub + 1)],
                                ident[:],
                            )
                        for sub in range(4):
                            ktile = 4 * c + sub
                            for h in range(HP):
                                nc.vector.tensor_copy(
                                    v_sb[:, ktile, h, 0:64],
                                    ps_t[
                                        :,
                                        128 * sub + 64 * h : 128 * sub
                                        + 64 * (h + 1),
                                    ],
                                )

            def attention(b, qt):
                ch = b * QT_PER_S + qt
                nkt = _nkt(qt, mode)
                av = [
                    ps512.tile([128, 512], F32, tag="ps512", name=f"av{ch}_{h}")
                    for h in range(HP)
                ]
                for sp in range(nkt // 2):
                    kts = (2 * sp, 2 * sp + 1)
                    ps_s = [
                        ps1024.tile(
                            [128, 1024], F32, tag="ps1024", name=f"s{ch}_{sp}_{h}"
                        )
                        for h in range(HP)
                    ]
                    for i, kt in enumerate(kts):
                        c, ks = b * QT_PER_S + kt // 4, kt % 4
                        for h in range(HP):
                            nc.tensor.matmul(
                                ps_s[h][:, 512 * i : 512 * (i + 1)],
                                kT_sb[
                                    64 * h : 64 * (h + 1),
                                    c,
                                    128 * ks : 128 * (ks + 1),
                                ],
                                qT_sb[64 * h : 64 * (h + 1), ch, :],
                                start=True,
                                stop=True,
                            )
                    p_sb = []
                    for h in range(HP):
                        pt = pp.tile([128, 1024], BF16, tag="p")
                        nc.scalar.activation(
                            pt[:], ps_s[h][:], AF.Exp, scale=float(SCALE)
                        )
                        p_sb.append(pt)
                    # masked tiles get the 0/1 mask applied into a separate
                    # tile (pure producer->consumer dep for the AV matmul).
                    def mask_index(kt):
                        if mode == "causal":
                            off = kt - 4 * qt
                            return off if 0 <= off < 4 else None
                        if mode == "general":
                            return qt * KT_PER_S + kt
                        return None

                    av_src = {}
                    for i, kt in enumerate(kts):
                        mi = mask_index(kt)
                        if mi is None:
                            for h in range(HP):
                                av_src[(i, h)] = p_sb[h][
                                    :, 512 * i : 512 * (i + 1)
                                ]
                        else:
                            for h in range(HP):
                                pm = pp.tile([128, 512], BF16, tag="pm", bufs=4)
                                nc.vector.tensor_tensor(
                                    pm[:],
                                    p_sb[h][:, 512 * i : 512 * (i + 1)],
                                    mt_sb[:, mi, :],
                                    mybir.AluOpType.mult,
                                )
                                av_src[(i, h)] = pm[:]
                    for i, kt in enumerate(kts):
                        for h in range(HP):
                            nc.tensor.matmul(
                                av[h][0:65, :],
                                v_sb[:, b * KT_PER_S + kt, h, 0:65],
                                av_src[(i, h)],
                                start=(kt == 0),
                                stop=(kt == nkt - 1),
                            )
                # epilogue: normalize (approx reciprocal, DRAM-replicate
                # broadcast across partitions) and stage the a2a shards.
                attnT = misc.tile([128, 512], F32R, tag="attnT", bufs=6)
                for h in range(HP):
                    rec = misc.tile([1, 512], F32, tag="r", bufs=4)
                    nc.vector.reciprocal(rec[:], av[h][64:65, :])
                    nc.sync.dma_start(r_dram[ch, h, :, :], rec[:])
                    rb = misc.tile([64, 512], F32, tag="rb", bufs=3)
                    nc.sync.dma_start(
                        rb[:], r_dram[ch, h, :, :].to_broadcast((64, 512))
                    )
                    nc.vector.tensor_tensor(
                        attnT[64 * h : 64 * (h + 1), :],
                        av[h][0:64, :],
                        rb[:],
                        mybir.AluOpType.mult,
                    )
                if skip_phase3:
                    return
                nc.sync.dma_start(a2a_in[b][2 * qt, :, :], attnT[:, 0:256])
                nc.sync.dma_start(
                    a2a_in[b][2 * qt + 1, :, :], attnT[:, 256:512]
                )

            def a2a(b):
                nc.gpsimd.collective_compute(
                    "AllToAll",
                    mybir.AluOpType.bypass,
                    replica_groups=[list(range(NCORES))],
                    ins=[a2a_in[b].opt()],
                    outs=[a2a_out[b].opt()],
                )

            def outproj(b):
                ab = const.tile([128, 8, 256], F32R, name=f"attn_all{b}")
                attn_all[b] = ab
                nc.sync.dma_start(ab[:], a2a_out[b].rearrange("r p q -> p r q"))
                for st2 in range(2):
                    for half in range(2):
                        pso = ps512.tile(
                            [128, 512], F32, tag="ps512", name=f"o{b}_{st2}_{half}"
                        )
                        for r in range(NCORES):
                            nc.tensor.matmul(
                                pso[:],
                                ab[:, r, 128 * st2 : 128 * (st2 + 1)],
                                wo_sb[:, r, 512 * half : 512 * (half + 1)],
                                start=(r == 0),
                                stop=(r == NCORES - 1),
                            )
                        osb = stage.tile([128, 512], F32, tag="osb", bufs=6)
                        nc.vector.tensor_copy(osb[:], pso[:])
                        nc.sync.dma_start(
                            out[
                                256 * b + 128 * st2 : 256 * b + 128 * (st2 + 1),
                                512 * half : 512 * (half + 1),
                            ],
                            osb[:],
                        )

            # ---- emission: interleave projection chunks with attention so
            # the PE stream stays dense and ACT starts early; the two A2As
            # overlap with batch-1 attention / the output projections.
            qkv_chunk(0)
            qkv_chunk(1)
            attention(0, 0)
            qkv_chunk(2)
            attention(0, 1)
            qkv_chunk(3)
            attention(0, 2)
            qkv_chunk(4)
            attention(0, 3)
            qkv_chunk(5)
            if not skip_phase3:
                a2a(0)
            attention(1, 0)
            qkv_chunk(6)
            attention(1, 1)
            qkv_chunk(7)
            attention(1, 2)
            attention(1, 3)
            if not skip_phase3:
                wo_sb = const.tile([128, 8, DM], F32R)
                nc.sync.dma_start(
                    wo_sb[:], wo.rearrange("(o p) n -> p o n", p=128)
                )
                outproj(0)
                a2a(1)
                outproj(1)
            else:
                dbg = stage.tile([128, 512], F32, tag="osb")
                nc.vector.tensor_copy(dbg[:], qT_sb[:, 0, :])
                nc.sync.dma_start(out[0:128, 0:512], dbg[:])
    _split_waits(nc)

    # The libneuronxla NEFF cache hashes the HLO, but the BIR travels in
    # backend_config which is NOT part of the hash — two different kernels
    # with identical I/O signatures collide and the stale NEFF runs. Encode
    # a hash of the BIR into the shape of an unused dummy input so the HLO
    # (and therefore the cache key) changes whenever the kernel changes.
    import hashlib

    hv = int.from_bytes(
        hashlib.sha256(nc.to_json_bytes()).digest()[:4], "little"
    )
    nonce_shape = [hv % 1021 + 1, (hv // 1021) % 1021 + 1]
    nc.dram_tensor("nonce", nonce_shape, F32, kind="ExternalInput")
    nc._nonce_shape = nonce_shape
    return nc


_BUILD_CACHE = {}


def _get_nc(mode, n_mask_tiles):
    key = (mode, n_mask_tiles)
    if key not in _BUILD_CACHE:
        _BUILD_CACHE[key] = build(mode, n_mask_tiles)
    return _BUILD_CACHE[key]


def kernel(x, Wqkv, Wo, mask):
    x = np.asarray(x)
    Wqkv = np.asarray(Wqkv)
    Wo = np.asarray(Wo)
    mask = np.asarray(mask)

    m2 = mask.reshape(S, S)
    if np.array_equal(m2, np.tril(np.ones((S, S), bool))):
        mode = "causal"
    elif m2.all():
        mode = "full"
    else:
        mode = "general"

    # host-side input prep: transpose x, slice per-head weight shards
    xT = np.ascontiguousarray(x.reshape(T, DM).T)
    w4 = Wqkv.reshape(DM, H, 3, D)

    if mode == "causal":
        # mask tile for diagonal offset o: [k=128, q=512], 1 where q >= k + 128*o
        qq = np.arange(512)[None, :]
        kk = np.arange(128)[:, None]
        mts = np.stack(
            [(qq - kk >= 128 * o) for o in range(4)]
        ).astype(ml_dtypes.bfloat16)
        n_mask_tiles = 4
    elif mode == "general":
        tiles = []
        for qt in range(QT_PER_S):
            for kt in range(KT_PER_S):
                sub = m2[512 * qt : 512 * (qt + 1), 128 * kt : 128 * (kt + 1)]
                tiles.append(sub.T)
        mts = np.stack(tiles).astype(ml_dtypes.bfloat16)
        n_mask_tiles = len(tiles)
    else:
        mts = None
        n_mask_tiles = 0

    nc = _get_nc(mode, n_mask_tiles)

    in_maps = []
    for j in range(NCORES):
        hs = slice(HP * j, HP * (j + 1))
        im = {
            "xT": xT,
            "wq": np.ascontiguousarray(
                w4[:, hs, 0, :].reshape(DM, HP * D)
            ),
            "wk": np.ascontiguousarray(
                w4[:, hs, 1, :].reshape(DM, HP * D)
            ),
            "wv": np.ascontiguousarray(
                w4[:, hs, 2, :].reshape(DM, HP * D)
            ),
            "wo": Wo,
            "nonce": np.zeros(nc._nonce_shape, np.float32),
        }
        if n_mask_tiles:
            im["mt"] = mts
        in_maps.append(im)

    res = run_bass_kernel_spmd(nc, in_maps, list(range(NCORES)))
    # core j's output rows: [0:256] = batch 0 tokens [256j, 256j+256),
    #                       [256:512] = batch 1 tokens [256j, 256j+256)
    full = np.empty((B, S, DM), np.float32)
    for j in range(NCORES):
        o = res.results[j]["out"]
        for b in range(B):
            full[b, 256 * j : 256 * (j + 1), :] = o[256 * b : 256 * (b + 1)]
    return full


if __name__ == "__main__":
    rng = np.random.default_rng(0)
    x = rng.standard_normal((B, S, DM), dtype=np.float32)
    Wqkv = rng.standard_normal((DM, 3 * H * D), dtype=np.float32) * DM**-0.5
    Wo = rng.standard_normal((H * D, DM), dtype=np.float32) * (H * D) ** -0.5
    mask = np.tril(np.ones((S, S), bool))[None, None]
    out = kernel(x=x, Wqkv=Wqkv, Wo=Wo, mask=mask)
    print(out.shape, out.dtype)



# revision 5
# speedup vs baseline: 1.4139x; 1.4139x over previous
"""Multi-head causal attention on 8 Trainium2 NeuronCores.

Sharding: tensor-parallel over heads (2 heads/core). Each core computes QKV
projection + attention for its 2 heads over all 4096 tokens, then a PARTIAL
output projection against its 128 rows of Wo (bf16). The all-reduce over the
8 partial outputs happens on the host — no on-device collective at all, so
cores are fully decoupled (launch skew and per-core clock-throttle skew no
longer serialize through a collective barrier).

Layout strategy (per core):
  - everything bf16 on device (host pre-casts x/weights); PSUM accumulation
    stays fp32 so only input quantization error is added (~0.4%).
  - qkvT = W^T @ x^T computed in transposed [feature, token] layout
    (x^T prepared on host) so attention contractions need no transposes of
    Q/K.
  - scores^T[k, q] = K_tile^T.T @ Q^T with the two heads row-tiled on the PE
    (d=64 contraction each, partitions 0-63 / 64-127, concurrent).
  - softmax: exp on ScalarE straight out of PSUM with the 1/sqrt(D) scale
    folded into the activation's free affine; no max-subtraction (scores are
    O(6) so exp is safe); causal masking is a multiplicative 0/1 bf16 mask on
    the diagonal tiles only, with the column extent restricted to the
    not-fully-masked range [128*o, 512).
  - AV: av^T[d, q] accumulated over k-tiles in PSUM; lhsT = [V_k | 1] needs V
    in natural [token, d] layout, produced by PE transpose. The ones column
    gives the softmax denominator on psum row 64 for free.
  - normalize: reciprocal_approx_fast on the denominator row (single DVE op,
    ~5x faster than iterative reciprocal), broadcast across partitions by
    GpSimd partition_broadcast (idle engine; no DRAM bounce), one fused DVE
    multiply writes the bf16 attention tile.
  - output projection per (batch, q-chunk) right after that chunk's attention
    so the PE stream stays dense; partial out rows DMA'd as bf16.
"""

import os

import numpy as np
import ml_dtypes

import concourse.bass as bass
import concourse.mybir as mybir
import concourse.tile as tile
from concourse.bass_utils import run_bass_kernel_spmd
from concourse.masks import make_identity
from concourse.vector_clock import ScopedClock

F32 = mybir.dt.float32
BF16 = mybir.dt.bfloat16
AF = mybir.ActivationFunctionType


def _install_cache_nonce_hook():
    """The libneuronxla NEFF cache hashes the HLO but the BIR rides in
    backend_config (excluded from the hash), so edited kernels with the same
    I/O signature can silently hit a stale cached NEFF. Inject a hash of the
    BIR into mhlo.frontend_attributes — which IS part of the model hash —
    the same way bass2jax ships the DVE tables."""
    import hashlib
    import concourse.bass2jax as bass2jax
    from jax.interpreters import mlir

    if getattr(bass2jax, "_ant_cache_nonce_hooked", False):
        return
    bass2jax._ant_cache_nonce_hooked = True
    orig = bass2jax._accumulate_module_dve_attrs

    def patched(ctx, nc):
        orig(ctx, nc)
        op = ctx.module_context.module.operation
        cur = (
            op.attributes["mhlo.frontend_attributes"]
            if "mhlo.frontend_attributes" in op.attributes
            else None
        )
        existing = (
            {a.name: mlir.ir.StringAttr(a.attr).value for a in cur}
            if cur is not None
            else {}
        )
        existing["ant.cache_nonce"] = hashlib.sha256(
            nc.to_json_bytes()
        ).hexdigest()
        op.attributes["mhlo.frontend_attributes"] = mlir.ir.DictAttr.get(
            {k: mlir.ir.StringAttr.get(v) for k, v in existing.items()}
        )

    bass2jax._accumulate_module_dve_attrs = patched


_install_cache_nonce_hook()


B, S, DM = 2, 2048, 1024
H, D = 16, 64
NCORES = 8
HP = H // NCORES          # heads per core
T = B * S                 # 4096 tokens
NCH = T // 512            # 8 token chunks of 512
KT_PER_S = S // 128       # 16 k-tiles per sequence
QT_PER_S = S // 512       # 4 q-tiles per sequence
SCALE = 1.0 / np.sqrt(D)

# env-tunable dev flags (defaults are the shipping config)
PBCAST = os.environ.get("ATT_PBCAST", "1") == "1"

MAX_WAITS = 1  # walrus in this container rejects >1 sem-wait per instruction


def _split_waits(nc, limit=MAX_WAITS):
    """Post-pass: move excess sem-waits onto preceding same-engine nops.

    Engines dispatch in program order and a sem-wait stalls the engine's NX
    before anything later is enqueued, so carrying the waits on nops placed
    immediately before the instruction is semantically identical.
    """
    n_id = 0
    for bb in nc.main_func.blocks:
        new = []
        for inst in bb.instructions:
            si = getattr(inst, "sync_info", None)
            if si is not None and len(si.on_wait) > limit:
                waits = list(si.on_wait)
                for i in range(0, len(waits) - limit, limit):
                    nop = mybir.InstNoOp(
                        name=f"wsplit-{n_id}", ins=[], outs=[], engine=inst.engine
                    )
                    n_id += 1
                    nop.sync_info = mybir.SyncInfo(
                        on_wait=waits[i : i + limit], on_update=[]
                    )
                    new.append(nop)
                kept = waits[len(waits) - limit :]
                inst.sync_info = mybir.SyncInfo(
                    on_wait=kept, on_update=list(si.on_update)
                )
            new.append(inst)
        bb.instructions = new


class _TileCtx(tile.TileContext):
    """Work around a walrus codegen limit: the stock tail drain carries one
    sem-wait per (engine, DMA-lane), but this compiler build rejects >1-2
    waits on a Drain ("Too many sync wait commands"). Put each wait on its
    own SP nop between the drain and the final barrier instead."""

    def _drain_and_barrier(self, tick_clock, wait_clock):
        nc = self.nc
        drain_inst = nc.sync.drain()
        wait_clock.add_sem_waits(
            drain_inst.ins, ScopedClock({None: tick_clock.global_clock})
        )
        si = drain_inst.ins.sync_info
        if si is not None and len(si.on_wait) > 1:
            waits = list(si.on_wait)
            drain_inst.ins.sync_info = mybir.SyncInfo(
                on_wait=[waits[0]], on_update=list(si.on_update)
            )
            for w in waits[1:]:
                nop = nc.sync.nop(nofuse=True, hint="tail_drain_wait_split")
                nop.ins.sync_info = mybir.SyncInfo(on_wait=[w], on_update=[])

        nc.all_engine_barrier()
        assert self.sems is not None
        popped = nc._tile_sem_poison_stack.pop()
        assert popped is self._sem_poison
        nc.clear_and_free_semaphores(list(self.sems.allocated().values()))
        nc.all_engine_barrier()


def _nkt(qt, mode):
    """Number of k-tiles attended by q-tile qt (within one sequence)."""
    return 4 * (qt + 1) if mode == "causal" else KT_PER_S


def build(mode, n_mask_tiles, skip_phase3=False):
    """Build the SPMD Bass program. mode: 'causal' | 'full' | 'general'."""
    nc = bass.Bass()

    xT = nc.dram_tensor("xT", [DM, T], BF16, kind="ExternalInput")
    wq = nc.dram_tensor("wq", [DM, 128], BF16, kind="ExternalInput")
    wk = nc.dram_tensor("wk", [DM, 128], BF16, kind="ExternalInput")
    wv = nc.dram_tensor("wv", [DM, 128], BF16, kind="ExternalInput")
    wo = nc.dram_tensor("wo", [128, DM], BF16, kind="ExternalInput")
    if n_mask_tiles:
        mt = nc.dram_tensor(
            "mt", [n_mask_tiles, 128, 512], BF16, kind="ExternalInput"
        )
    out = nc.dram_tensor("out", [T, DM], BF16, kind="ExternalOutput")

    with _TileCtx(nc) as tc:
        with (
            tc.tile_pool(name="const", bufs=1) as const,
            tc.tile_pool(name="xin", bufs=2) as xin,
            tc.tile_pool(name="stage", bufs=3) as stage,
            tc.tile_pool(name="pp", bufs=6) as pp,
            tc.tile_pool(name="misc", bufs=4) as misc,
            tc.tile_pool(name="ps512", bufs=4, space="PSUM") as ps512,
            tc.tile_pool(name="ps1024", bufs=2, space="PSUM") as ps1024,
            tc.tile_pool(name="dram", bufs=1, space="DRAM") as dram,
        ):
            # ---- resident SBUF tensors ----
            wq_sb = const.tile([128, 8, 128], BF16)
            wk_sb = const.tile([128, 8, 128], BF16)
            wv_sb = const.tile([128, 8, 128], BF16)
            nc.sync.dma_start(wq_sb[:], wq.rearrange("(o p) e -> p o e", p=128))
            nc.sync.dma_start(wk_sb[:], wk.rearrange("(o p) e -> p o e", p=128))
            nc.sync.dma_start(wv_sb[:], wv.rearrange("(o p) e -> p o e", p=128))
            wo_sb = const.tile([128, DM], BF16)
            nc.sync.dma_start(wo_sb[:], wo[:, :])

            if n_mask_tiles:
                mt_sb = const.tile([128, n_mask_tiles, 512], BF16)
                nc.sync.dma_start(mt_sb[:], mt.rearrange("m p q -> p m q"))

            qT_sb = const.tile([128, NCH, 512], BF16)
            kT_sb = const.tile([128, NCH, 512], BF16)
            # V in [token, feature] layout, per k-tile, per head:
            # [p=token%128, ktile, head, 80] where cols 0:64 = v, col 64 = 1.0
            v_sb = const.tile([128, T // 128, HP, 80], BF16)
            nc.vector.memset(v_sb[:, :, :, 64:65], 1.0)
            ident = const.tile([128, 128], BF16)
            make_identity(nc, ident[:])

            den_dram = dram.tile([NCH, HP, 1, 512], F32)
            rec_dram = dram.tile([NCH, HP, 1, 512], F32)

            def qkv_chunk(c):
                xt = xin.tile([128, 8, 512], BF16, tag="xt", name=f"xt{c}")
                nc.sync.dma_start(
                    xt[:],
                    xT[:, 512 * c : 512 * (c + 1)].rearrange(
                        "(o p) s -> p o s", p=128
                    ),
                )
                for name, w_sb, dst in (
                    ("q", wq_sb, qT_sb),
                    ("k", wk_sb, kT_sb),
                    ("v", wv_sb, None),
                ):
                    psum = ps512.tile(
                        [128, 512], F32, tag="ps512", name=f"ps_{name}{c}"
                    )
                    for kt in range(8):
                        nc.tensor.matmul(
                            psum[:],
                            w_sb[:, kt, :],
                            xt[:, kt, :],
                            start=(kt == 0),
                            stop=(kt == 7),
                        )
                    if dst is not None:
                        nc.vector.tensor_copy(dst[:, c, :], psum[:])
                    else:
                        # PE-based transpose of V into [token, feature] layout
                        vstg = stage.tile([128, 512], BF16, tag="vstg")
                        nc.vector.tensor_copy(vstg[:], psum[:])
                        ps_t = ps512.tile(
                            [128, 512], BF16, tag="ps512", name=f"ps_t{c}"
                        )
                        for sub in range(4):
                            nc.tensor.transpose(
                                ps_t[:, 128 * sub : 128 * (sub + 1)],
                                vstg[:, 128 * sub : 128 * (sub + 1)],
                                ident[:],
                            )
                        for sub in range(4):
                            ktile = 4 * c + sub
                            for h in range(HP):
                                nc.vector.tensor_copy(
                                    v_sb[:, ktile, h, 0:64],
                                    ps_t[
                                        :,
                                        128 * sub + 64 * h : 128 * sub
                                        + 64 * (h + 1),
                                    ],
                                )

            def attention(b, qt):
                ch = b * QT_PER_S + qt
                nkt = _nkt(qt, mode)

                # diagonal-tile bookkeeping: mask index + valid column start
                def mask_index(kt):
                    if mode == "causal":
                        off = kt - 4 * qt
                        return off if 0 <= off < 4 else None
                    if mode == "general":
                        return qt * KT_PER_S + kt
                    return None

                def col0(kt):
                    # first not-fully-masked q column of this k-tile
                    if mode == "causal":
                        off = kt - 4 * qt
                        if 0 <= off < 4:
                            return 128 * off
                    return 0

                av = [
                    ps512.tile([128, 512], F32, tag="ps512", name=f"av{ch}_{h}")
                    for h in range(HP)
                ]
                for sp in range(nkt // 2):
                    kts = (2 * sp, 2 * sp + 1)
                    ps_s = [
                        ps1024.tile(
                            [128, 1024], F32, tag="ps1024", name=f"s{ch}_{sp}_{h}"
                        )
                        for h in range(HP)
                    ]
                    for i, kt in enumerate(kts):
                        c, ks = b * QT_PER_S + kt // 4, kt % 4
                        for h in range(HP):
                            nc.tensor.matmul(
                                ps_s[h][:, 512 * i : 512 * (i + 1)],
                                kT_sb[
                                    64 * h : 64 * (h + 1),
                                    c,
                                    128 * ks : 128 * (ks + 1),
                                ],
                                qT_sb[64 * h : 64 * (h + 1), ch, :],
                                start=True,
                                stop=True,
                            )
                    p_sb = []
                    for h in range(HP):
                        pt = pp.tile([128, 1024], BF16, tag="p")
                        nc.scalar.activation(
                            pt[:], ps_s[h][:], AF.Exp, scale=float(SCALE)
                        )
                        p_sb.append(pt)
                    # masked tiles get the 0/1 mask applied into a separate
                    # tile, restricted to the not-fully-masked column range.
                    av_src = {}
                    for i, kt in enumerate(kts):
                        mi = mask_index(kt)
                        c0 = col0(kt)
                        if mi is None:
                            for h in range(HP):
                                av_src[(i, h)] = (
                                    p_sb[h][:, 512 * i : 512 * (i + 1)],
                                    0,
                                )
                        else:
                            for h in range(HP):
                                pm = pp.tile([128, 512], BF16, tag="pm", bufs=4)
                                nc.vector.tensor_tensor(
                                    pm[:, c0:512],
                                    p_sb[h][:, 512 * i + c0 : 512 * (i + 1)],
                                    mt_sb[:, mi, c0:512],
                                    mybir.AluOpType.mult,
                                )
                                av_src[(i, h)] = (pm[:, c0:512], c0)
                    for i, kt in enumerate(kts):
                        for h in range(HP):
                            src, c0 = av_src[(i, h)]
                            nc.tensor.matmul(
                                av[h][0:65, c0:512],
                                v_sb[:, b * KT_PER_S + kt, h, 0:65],
                                src,
                                start=(kt == 0),
                                stop=(kt == nkt - 1),
                            )
                # epilogue: normalize. DVE reciprocal costs 8 cyc per FREE
                # element (serial along the free dim), so the [1,512] row is
                # reshaped to [128,4] via a DRAM round-trip before the recip
                # (0.19us instead of 3.3us). The PSUM->SBUF row copy rides on
                # ScalarE (Copy lives in every ACT table set — no reload).
                attnT = misc.tile([128, 512], BF16, tag="attnT", bufs=6)
                for h in range(HP):
                    den = misc.tile([1, 512], F32, tag="den", bufs=4)
                    nc.scalar.copy(den[:], av[h][64:65, :])
                    nc.sync.dma_start(den_dram[ch, h, :, :], den[:])
                    den4 = misc.tile([128, 4], F32, tag="den4", bufs=4)
                    nc.sync.dma_start(
                        den4[:],
                        den_dram[ch, h, 0, :].rearrange("(p f) -> p f", p=128),
                    )
                    rec4 = misc.tile([128, 4], F32, tag="rec4", bufs=4)
                    nc.vector.reciprocal(rec4[:], den4[:])
                    nc.sync.dma_start(
                        rec_dram[ch, h, 0, :].rearrange("(p f) -> p f", p=128),
                        rec4[:],
                    )
                    rb = misc.tile([64, 512], F32, tag="rb", bufs=3)
                    nc.sync.dma_start(
                        rb[:], rec_dram[ch, h, :, :].to_broadcast((64, 512))
                    )
                    nc.vector.tensor_tensor(
                        attnT[64 * h : 64 * (h + 1), :],
                        av[h][0:64, :],
                        rb[:],
                        mybir.AluOpType.mult,
                    )
                return attnT

            def outproj(b, qt, attnT):
                # partial output projection for this chunk's 512 tokens:
                # out[tok, :] += attnT[:, tok].T @ Wo[128 core rows, :]
                row0 = 2048 * b + 512 * qt
                for tt in range(4):
                    for half in range(2):
                        pso = ps512.tile(
                            [128, 512],
                            F32,
                            tag="ps512",
                            name=f"o{b}_{qt}_{tt}_{half}",
                        )
                        nc.tensor.matmul(
                            pso[:],
                            attnT[:, 128 * tt : 128 * (tt + 1)],
                            wo_sb[:, 512 * half : 512 * (half + 1)],
                            start=True,
                            stop=True,
                        )
                        osb = stage.tile([128, 512], BF16, tag="osb", bufs=6)
                        if half == 0:
                            nc.vector.tensor_copy(osb[:], pso[:])
                        else:
                            nc.scalar.copy(osb[:], pso[:])
                        nc.sync.dma_start(
                            out[
                                row0 + 128 * tt : row0 + 128 * (tt + 1),
                                512 * half : 512 * (half + 1),
                            ],
                            osb[:],
                        )

            # ---- emission: interleave projection chunks, attention and the
            # per-chunk output projections so the PE stream stays dense.
            qkv_chunk(0)
            qkv_chunk(1)
            a = attention(0, 0)
            qkv_chunk(2)
            outproj(0, 0, a)
            a = attention(0, 1)
            qkv_chunk(3)
            outproj(0, 1, a)
            a = attention(0, 2)
            qkv_chunk(4)
            outproj(0, 2, a)
            a = attention(0, 3)
            qkv_chunk(5)
            outproj(0, 3, a)
            a = attention(1, 0)
            qkv_chunk(6)
            outproj(1, 0, a)
            a = attention(1, 1)
            qkv_chunk(7)
            outproj(1, 1, a)
            a = attention(1, 2)
            outproj(1, 2, a)
            a = attention(1, 3)
            outproj(1, 3, a)
    _split_waits(nc)

    # The libneuronxla NEFF cache hashes the HLO, but the BIR travels in
    # backend_config which is NOT part of the hash — two different kernels
    # with identical I/O signatures collide and the stale NEFF runs. Encode
    # a hash of the BIR into the shape of an unused dummy input so the HLO
    # (and therefore the cache key) changes whenever the kernel changes.
    import hashlib

    hv = int.from_bytes(
        hashlib.sha256(nc.to_json_bytes()).digest()[:4], "little"
    )
    nonce_shape = [hv % 1021 + 1, (hv // 1021) % 1021 + 1]
    nc.dram_tensor("nonce", nonce_shape, F32, kind="ExternalInput")
    nc._nonce_shape = nonce_shape
    return nc


_BUILD_CACHE = {}


def _get_nc(mode, n_mask_tiles):
    key = (mode, n_mask_tiles)
    if key not in _BUILD_CACHE:
        _BUILD_CACHE[key] = build(mode, n_mask_tiles)
    return _BUILD_CACHE[key]


def kernel(x, Wqkv, Wo, mask):
    x = np.asarray(x)
    Wqkv = np.asarray(Wqkv)
    Wo = np.asarray(Wo)
    mask = np.asarray(mask)

    m2 = mask.reshape(S, S)
    if np.array_equal(m2, np.tril(np.ones((S, S), bool))):
        mode = "causal"
    elif m2.all():
        mode = "full"
    else:
        mode = "general"

    BF = ml_dtypes.bfloat16
    # host-side input prep: transpose+cast x, slice per-head weight shards
    xT = np.ascontiguousarray(x.reshape(T, DM).T.astype(BF))
    w4 = Wqkv.reshape(DM, H, 3, D)

    if mode == "causal":
        # mask tile for diagonal offset o: [k=128, q=512], 1 where q >= k + 128*o
        qq = np.arange(512)[None, :]
        kk = np.arange(128)[:, None]
        mts = np.stack(
            [(qq - kk >= 128 * o) for o in range(4)]
        ).astype(BF)
        n_mask_tiles = 4
    elif mode == "general":
        tiles = []
        for qt in range(QT_PER_S):
            for kt in range(KT_PER_S):
                sub = m2[512 * qt : 512 * (qt + 1), 128 * kt : 128 * (kt + 1)]
                tiles.append(sub.T)
        mts = np.stack(tiles).astype(BF)
        n_mask_tiles = len(tiles)
    else:
        mts = None
        n_mask_tiles = 0

    nc = _get_nc(mode, n_mask_tiles)

    in_maps = []
    for j in range(NCORES):
        hs = slice(HP * j, HP * (j + 1))
        im = {
            "xT": xT,
            "wq": np.ascontiguousarray(
                w4[:, hs, 0, :].reshape(DM, HP * D).astype(BF)
            ),
            "wk": np.ascontiguousarray(
                w4[:, hs, 1, :].reshape(DM, HP * D).astype(BF)
            ),
            "wv": np.ascontiguousarray(
                w4[:, hs, 2, :].reshape(DM, HP * D).astype(BF)
            ),
            "wo": np.ascontiguousarray(
                Wo[128 * j : 128 * (j + 1), :].astype(BF)
            ),
            "nonce": np.zeros(nc._nonce_shape, np.float32),
        }
        if n_mask_tiles:
            im["mt"] = mts
        in_maps.append(im)

    res = run_bass_kernel_spmd(nc, in_maps, list(range(NCORES)))
    # host all-reduce of the 8 partial projections
    acc = np.zeros((T, DM), np.float32)
    for j in range(NCORES):
        acc += res.results[j]["out"].astype(np.float32)
    return acc.reshape(B, S, DM)


if __name__ == "__main__":
    rng = np.random.default_rng(0)
    x = rng.standard_normal((B, S, DM), dtype=np.float32)
    Wqkv = rng.standard_normal((DM, 3 * H * D), dtype=np.float32) * DM**-0.5
    Wo = rng.standard_normal((H * D, DM), dtype=np.float32) * (H * D) ** -0.5
    mask = np.tril(np.ones((S, S), bool))[None, None]
    out = kernel(x=x, Wqkv=Wqkv, Wo=Wo, mask=mask)
    print(out.shape, out.dtype)
